# revision 1
# baseline (speedup 1.0000x reference)
"""Trainium2 Bass kernel for LocalDenseConv2D + BatchNorm + PReLU.

Problem (hardcoded shapes):
  x:      (8, 64, 64, 256)  f32   (B, IN_C, L, T)
  weight: (576, 64, 64)     f32   (K = IN_C*9, OUT_C, OUT_L)  k = ci*9 + i*3 + j
  bias:   (64, 64)          f32   (OUT_C, OUT_L)
  gamma, beta: (64,)        f32
  alpha:  (1,)              f32
  out:    (8, 64, 64, 256)  f32

Sharding: out_l across 8 cores (8 rows each), all batches per core.
Per core the conv is 288 f32r matmuls (K=64 per (i,j) tap, M=64 channels,
N=512 over (batch-pair, t)), with the two batch-halves row-paired on PE
row groups 0-63 / 64-127 for 2x concurrency.  BatchNorm stats: local
bn_stats/bn_aggr per channel, AllGather of per-core (mean, var), exact
global merge via bn_aggr on reconstructed (count, mean, M2) triples.
BN-apply + PReLU is a single scalar-engine Prelu activation with
per-partition scale/bias APs.
"""
import sys
import numpy as np

if '/opt/trn_rl_repo' not in sys.path:
    sys.path.insert(0, '/opt/trn_rl_repo')

import concourse.bass as bass
import concourse.bacc as bacc
import concourse.mybir as mybir
import concourse.tile as tile
from concourse.bass_utils import run_bass_kernel_spmd

F32 = mybir.dt.float32
F32R = mybir.dt.float32r
AF = mybir.ActivationFunctionType

B, IN_C, L, T = 8, 64, 64, 256
OUT_C, OUT_L = 64, 64
NCORES = 8
L_LOC = L // NCORES          # 8 out_l rows per core
SLAB = L_LOC + 2             # 10 x-rows incl. halo
TP = T + 2                   # padded t
EPS = 1e-5
N_LOCAL = L_LOC * 2 * 4 * T  # elems per channel per core = 16384
N_GLOBAL = B * L * T         # 131072

_cache = {}


def _build():
    nc = bacc.Bacc("TRN2", target_bir_lowering=False, debug=False,
                   num_devices=NCORES)
    xr = nc.dram_tensor("xr", [128, 4, SLAB, TP], F32R, kind="ExternalInput")
    wr = nc.dram_tensor("wr", [128, 9, L_LOC, OUT_C], F32R, kind="ExternalInput")
    br = nc.dram_tensor("br", [OUT_C, L_LOC], F32, kind="ExternalInput")
    gr = nc.dram_tensor("gr", [OUT_C, 1], F32, kind="ExternalInput")
    er = nc.dram_tensor("er", [OUT_C, 1], F32, kind="ExternalInput")
    ar = nc.dram_tensor("ar", [OUT_C, 1], F32, kind="ExternalInput")
    yo = nc.dram_tensor("yo", [OUT_C, L_LOC, 2, 4, T], F32, kind="ExternalOutput")

    cc_in = nc.dram_tensor("cc_in", [OUT_C, 2], F32)
    cc_out = nc.dram_tensor("cc_out", [NCORES * OUT_C, 2], F32, addr_space="Shared")

    with tile.TileContext(nc) as tc:
        with (
            tc.tile_pool(name="const", bufs=1) as cpool,
            tc.tile_pool(name="xp", bufs=1) as xpool,
            tc.tile_pool(name="op", bufs=1) as opool,
            tc.tile_pool(name="fp", bufs=3) as fpool,
            tc.tile_pool(name="ps", bufs=2, space="PSUM") as ppool,
        ):
            wt = cpool.tile([128, 9, L_LOC, OUT_C], F32R)
            bt = cpool.tile([OUT_C, L_LOC], F32)
            gt = cpool.tile([OUT_C, 1], F32)
            et = cpool.tile([OUT_C, 1], F32)
            at = cpool.tile([OUT_C, 1], F32)
            xt = xpool.tile([128, 4, SLAB, TP], F32R)
            ot = opool.tile([OUT_C, L_LOC, 2, 1024], F32)
            stats = cpool.tile([OUT_C, L_LOC, 2, 2, 6], F32)

            # weights + small consts first, then x rows so matmuls start early
            nc.sync.dma_start(wt[:], wr.ap())
            nc.sync.dma_start(bt[:], br.ap())
            nc.sync.dma_start(gt[:], gr.ap())
            nc.sync.dma_start(et[:], er.ap())
            nc.sync.dma_start(at[:], ar.ap())
            for s in range(SLAB):
                nc.sync.dma_start(xt[:, :, s, :], xr.ap()[:, :, s, :])

            # ---- conv ----
            for lp in range(L_LOC):
                p0 = ppool.tile([64, 1024], F32, tag="p0")
                p1 = ppool.tile([64, 1024], F32, tag="p1")
                pt = (p0, p1)
                for nt in range(2):
                    for combo in range(9):
                        di, dj = combo // 3, combo % 3
                        first = combo == 0
                        last = combo == 8
                        for bh in range(2):
                            rhs = xt[64 * bh:64 * (bh + 1),
                                     2 * nt:2 * nt + 2,
                                     lp + di,
                                     dj:dj + T]
                            lhsT = wt[64 * bh:64 * (bh + 1), combo, lp, :]
                            nc.tensor.matmul(
                                pt[bh][:, 512 * nt:512 * (nt + 1)],
                                lhsT, rhs, start=first, stop=last)
                for bh in range(2):
                    nc.scalar.activation(ot[:, lp, bh, :], pt[bh][:, :],
                                         AF.Identity, bias=bt[:, lp:lp + 1])
                    for h in range(2):
                        nc.vector.bn_stats(
                            stats[:, lp, bh, h, :],
                            pt[bh][:, 512 * h:512 * (h + 1)])

            # ---- local stats -> collective -> global stats ----
            loc = cpool.tile([OUT_C, 2], F32)
            nc.vector.bn_aggr(loc[:], stats[:].rearrange("p a b c d -> p (a b c d)"))
            nc.sync.dma_start(cc_in.ap(), loc[:])
            nc.gpsimd.collective_compute(
                "AllGather", mybir.AluOpType.bypass,
                replica_groups=[list(range(NCORES))],
                ins=[cc_in[:]], outs=[cc_out[:]])
            gath = cpool.tile([OUT_C, NCORES, 2], F32)
            nc.sync.dma_start(
                gath[:],
                cc_out.ap().rearrange("(r c) s -> c r s", c=OUT_C))

            # rebuild bn_stats triples (count, mean, M2) per rank, merge
            tri = cpool.tile([OUT_C, NCORES, 3], F32)
            nc.vector.memset(tri[:], float(N_LOCAL))
            nc.scalar.activation(tri[:, :, 1], gath[:, :, 0], AF.Copy)
            nc.scalar.mul(tri[:, :, 2], gath[:, :, 1], float(N_LOCAL))
            gstat = cpool.tile([OUT_C, 2], F32)
            nc.vector.bn_aggr(gstat[:], tri[:].rearrange("p a b -> p (a b)"))

            # scale = gamma / sqrt(var + eps); shift = beta - mean * scale
            std = cpool.tile([OUT_C, 1], F32)
            rstd = cpool.tile([OUT_C, 1], F32)
            sca = cpool.tile([OUT_C, 1], F32)
            shi = cpool.tile([OUT_C, 1], F32)
            epst = cpool.tile([OUT_C, 1], F32)
            nc.vector.memset(epst[:], EPS)
            nc.scalar.activation(std[:], gstat[:, 1:2], AF.Sqrt, bias=epst[:])
            nc.vector.reciprocal(rstd[:], std[:])
            nc.vector.tensor_tensor(sca[:], gt[:], rstd[:], mybir.AluOpType.mult)
            nc.vector.tensor_tensor(shi[:], gstat[:, 0:1], sca[:], mybir.AluOpType.mult)
            nc.vector.tensor_tensor(shi[:], et[:], shi[:], mybir.AluOpType.subtract)

            # ---- fused BN-apply + PReLU + store ----
            for lp in range(L_LOC):
                fo = fpool.tile([OUT_C, 2048], F32, tag="fo")
                nc.scalar.activation(
                    fo[:], ot[:, lp, :, :].rearrange("p a n -> p (a n)"),
                    AF.Prelu, bias=shi[:], scale=sca[:], alpha=at[:])
                nc.sync.dma_start(
                    yo.ap()[:, lp, :, :, :].rearrange("p a b t -> p (a b t)"),
                    fo[:])
    nc.compile()
    return nc


def _prep(x, weight, bias, gamma, beta, alpha):
    """Build per-core input maps (host-side shard + relayout)."""
    in_maps = []
    # x relayout: [bh*64+ci, b4, slab_row, 1+t]
    xpad = np.zeros((B, IN_C, L + 2, TP), np.float32)
    xpad[:, :, 1:L + 1, 1:T + 1] = x
    for r in range(NCORES):
        l0 = r * L_LOC
        slab = xpad[:, :, l0:l0 + SLAB, :]          # (B, IN_C, SLAB, TP)
        xr = slab.reshape(2, 4, IN_C, SLAB, TP).transpose(0, 2, 1, 3, 4)
        xr = np.ascontiguousarray(xr.reshape(128, 4, SLAB, TP))
        # weight: [bh*64+ci, combo, l, c] = weight[ci*9+combo, c, l0+l]
        wl = weight[:, :, l0:l0 + L_LOC]            # (576, 64, 8)
        wl = wl.reshape(IN_C, 9, OUT_C, L_LOC).transpose(0, 1, 3, 2)  # ci,combo,l,c
        wr = np.ascontiguousarray(
            np.broadcast_to(wl[None], (2, IN_C, 9, L_LOC, OUT_C))
            .reshape(128, 9, L_LOC, OUT_C))
        br = np.ascontiguousarray(bias[:, l0:l0 + L_LOC])
        in_maps.append({
            "xr": xr, "wr": wr, "br": br,
            "gr": gamma.reshape(OUT_C, 1).astype(np.float32),
            "er": beta.reshape(OUT_C, 1).astype(np.float32),
            "ar": np.full((OUT_C, 1), float(alpha[0]), np.float32),
        })
    return in_maps


def kernel(x, weight, bias, gamma, beta, alpha, trace=False):
    x = np.asarray(x, np.float32)
    weight = np.asarray(weight, np.float32)
    bias = np.asarray(bias, np.float32)
    gamma = np.asarray(gamma, np.float32)
    beta = np.asarray(beta, np.float32)
    alpha = np.asarray(alpha, np.float32)

    if "nc" not in _cache:
        _cache["nc"] = _build()
    nc = _cache["nc"]
    in_maps = _prep(x, weight, bias, gamma, beta, alpha)
    res = run_bass_kernel_spmd(nc, in_maps, list(range(NCORES)), trace=trace)
    kernel._last = res

    out = np.empty((B, OUT_C, L, T), np.float32)
    for r in range(NCORES):
        yo = res.results[r]["yo"]                   # (64, 8, 2, 4, 256)
        l0 = r * L_LOC
        # out[bh*4+b4, c, l0+lp, t] = yo[c, lp, bh, b4, t]
        out[:, :, l0:l0 + L_LOC, :] = yo.transpose(2, 3, 0, 1, 4).reshape(
            B, OUT_C, L_LOC, T)
    return out

